# revision 73
# baseline (speedup 1.0000x reference)
"""BinaryDense kernel for Trainium2: out = sign(x) @ sign(w).

Full shapes: x [8192, 4096] f32, w [4096, 4096] f32 -> out [8192, 4096] f32.

Sharding over 8 NeuronCores (2D): x rows split 4 ways, w columns split 2 ways.
Each core computes a [2048, 2048] output block from x_shard [2048, 4096] and
w_shard [4096, 2048]. The host slices inputs and reassembles the output; no
collectives are needed.

Per-core kernel: binarize both operands on-chip to fp8e4 (+-1 is exact,
products are +-1 and sums are integers <= 4096, so fp32 PSUM accumulation is
exact), keep binarized w resident in SBUF, and run fp8 DoubleRow matmuls
(2 contraction tiles per pass).

Input handling:
  - Loads are SWDGE casting DMAs (f32 DRAM -> bf16 SBUF).  bf16 rounding
    cannot flip a sign (values below the smallest bf16 subnormal would need
    |x| < 5e-41; probability ~0 for randn inputs), so sign() is unaffected.
  - x tiles are PE-transposed directly out of the bf16 staging buffer; the
    sign binarization is FUSED into the PSUM->SBUF eviction (ACT Sign
    activation, bf16 psum -> fp8 SBUF).  No separate sign pass over x.
  - w signs alternate between ACT (1-op Sign activation) and DVE (2-op
    min/max-clamp sign) so no single engine paces the w pipeline.

Scheduling: w is loaded in two COLUMN-HALF passes.  DMA order is x0, the
first w half (interleaving x1), the remaining x blocks, then the second w
half — whose signs land during the dense phase, where ACT/DVE have slack.
Compute runs as two sweeps over the row blocks (one per w column-half):
sweep 1 starts as soon as the first half of w is signed (~35us in) and
carries all transposes (emitted TLOOK blocks ahead); each (row block,
column half) is a PE burst of 2x16 DoubleRow matmuls accumulating full K
into one psum bank per 512-wide chunk, evicted with one copy and written
out per half-row so staging tiles recycle immediately.  m-block 0's sweep-1
matmuls are emitted pair-major so they consume w tiles as they arrive.
"""

import numpy as np

import concourse.mybir as mybir
import concourse.tile as tile
from concourse import bacc
from concourse.bass_utils import run_bass_kernel_spmd
from concourse.masks import make_identity

P = 128
N_CORES = 8
RM, RN = 4, 2            # row shards of x, column shards of w
M_FULL, K, N_FULL = 8192, 4096, 4096
M_SH, N_SH = M_FULL // RM, N_FULL // RN   # 2048, 2048
KB = K // P              # 32 contraction tiles
MB = M_SH // P           # 16 row blocks
NB = N_SH // 512         # 4 psum-width column chunks
TLOOK = 2                # transpose lookahead (m-blocks) over matmuls
X_HEAD = 2               # x blocks loaded before w
SURF = 2                 # m-blocks whose matmuls surf the w load; <= X_HEAD
TGRP = 8                 # transposes sharing one psum tile (bf16: 1 bank)

USE_FP8_DR = True

F32 = mybir.dt.float32
BF16 = mybir.dt.bfloat16
FP8 = mybir.dt.float8e4
I16 = mybir.dt.int16

_NC_CACHE = None


def build_nc():
    mm_dt = FP8 if USE_FP8_DR else BF16

    nc = bacc.Bacc("TRN2", target_bir_lowering=False, debug=False,
                   num_devices=N_CORES)
    x = nc.dram_tensor("x", [M_SH, K], F32, kind="ExternalInput").ap()
    w = nc.dram_tensor("w", [K, N_SH], F32, kind="ExternalInput").ap()
    out = nc.dram_tensor("out", [M_SH, N_SH], I16, kind="ExternalOutput").ap()

    with tile.TileContext(nc) as tc:
        with (
            tc.tile_pool(name="const", bufs=1) as const_pool,
            tc.tile_pool(name="wbin", bufs=1) as wbin_pool,
            tc.tile_pool(name="xTr", bufs=1) as xT_pool,
            tc.tile_pool(name="ftmp", bufs=4) as ftmp_pool,
            tc.tile_pool(name="obuf", bufs=3) as obuf_pool,
            tc.tile_pool(name="psumT", bufs=3, space="PSUM") as psumT_pool,
            tc.tile_pool(name="psumO", bufs=5, space="PSUM") as psumO_pool,
        ):
            ident = const_pool.tile([P, P], BF16)
            make_identity(nc, ident)

            # Binarized, resident operands: w as [p, kb, n]; xT as [p, mb, kb, m]
            wbin = wbin_pool.tile([P, KB, N_SH], mm_dt)
            xT = xT_pool.tile([P, MB, KB, P], mm_dt)
            xstage = [None] * MB

            def load_x(mb):
                xt = ftmp_pool.tile([P, K], BF16, tag="xstage")
                nc.gpsimd.dma_start(out=xt[:], in_=x[mb * P:(mb + 1) * P, :])
                xstage[mb] = xt

            w3d = w.rearrange("(o p) n -> p o n", p=P)   # [128, KB, N_SH]
            NH = N_SH // 2

            def load_w(kb2, half):
                # Load one column-half of two k-tiles per DMA (0.5 MiB dest).
                nsl = slice(half * NH, (half + 1) * NH)
                wt = ftmp_pool.tile([P, 2, NH], BF16, tag="wstage")
                nc.gpsimd.dma_start(
                    out=wt[:], in_=w3d[:, 2 * kb2:2 * kb2 + 2, nsl])
                dst = wbin[:, 2 * kb2:2 * kb2 + 2, nsl]
                # w signs split between ACT (1-op Sign) and DVE (2-op clamp
                # sign: min(max(x*HUGE,-1),1), exact except |x| below the
                # smallest bf16 subnormal, probability ~0 for randn inputs;
                # sign(0)=0 is preserved).  The second half leans on DVE,
                # since ACT is saturated by transpose evictions during the
                # dense phase where those signs land.
                on_act = (kb2 % 8 < 5) if half == 0 else (kb2 % 4 == 0)
                if on_act:
                    nc.scalar.sign(dst, wt[:])
                else:
                    nc.vector.tensor_scalar(
                        dst, wt[:], 3.4e38, -1.0,
                        mybir.AluOpType.mult, mybir.AluOpType.max)
                    nc.vector.tensor_scalar(
                        dst, dst, 1.0, None, mybir.AluOpType.min)

            def transposes(mb, early=False):
                xt = xstage[mb]
                for g in range(KB // TGRP):
                    pt = psumT_pool.tile([P, TGRP, P], BF16, tag="psumT")
                    for j in range(TGRP):
                        kb = g * TGRP + j
                        nc.tensor.transpose(
                            pt[:, j, :], xt[:, kb * P:(kb + 1) * P], ident[:])
                    # Fused sign + downconvert during PSUM eviction.  The
                    # prologue blocks evict via the DVE clamp sign instead,
                    # since ACT is saturated by the first w-half signs then.
                    dst = xT[:, mb, g * TGRP:(g + 1) * TGRP, :]
                    if early:
                        nc.vector.tensor_scalar(
                            dst, pt[:], 3.4e38, -1.0,
                            mybir.AluOpType.mult, mybir.AluOpType.max)
                        nc.vector.tensor_scalar(
                            dst, dst, 1.0, None, mybir.AluOpType.min)
                    else:
                        nc.scalar.sign(dst, pt[:])

            def mm(po, mb, kb, nsl, start, stop):
                if USE_FP8_DR:
                    nc.tensor.matmul(
                        po[:], xT[:, mb, kb:kb + 2, :], wbin[:, kb:kb + 2, nsl],
                        start=start, stop=stop,
                        perf_mode=mybir.MatmulPerfMode.DoubleRow)
                else:
                    nc.tensor.matmul(
                        po[:], xT[:, mb, kb, :], wbin[:, kb, nsl],
                        start=start, stop=False)
                    nc.tensor.matmul(
                        po[:], xT[:, mb, kb + 1, :], wbin[:, kb + 1, nsl],
                        start=False, stop=stop)

            npair = KB // 2
            nsls = [slice(nb * 512, (nb + 1) * 512) for nb in range(NB)]

            def bass_ts(j):
                return slice(j * 512, (j + 1) * 512)

            def matmuls(mb, half, surf=False):
                # One column-half of one row block: 2 psum groups, one int16
                # staging tile (values are integers <= 4096, exactly
                # representable; the host widens back to f32), one out DMA.
                nbs = [2 * half, 2 * half + 1]
                ob = obuf_pool.tile([P, N_SH // 2], I16, tag="obuf")
                pos = [psumO_pool.tile([P, 512], F32, tag="psumO", name="po")
                       for _ in nbs]
                if surf:
                    # Pair-major emission: each arriving w pair immediately
                    # feeds the matmuls, so this block's matmuls overlap the
                    # w load instead of waiting for it.
                    for i in range(npair):
                        for j, nb in enumerate(nbs):
                            mm(pos[j], mb, 2 * i, nsls[nb],
                               start=(i == 0), stop=(i == npair - 1))
                else:
                    for j, nb in enumerate(nbs):
                        for i in range(npair):
                            mm(pos[j], mb, 2 * i, nsls[nb],
                               start=(i == 0), stop=(i == npair - 1))
                for j, nb in enumerate(nbs):
                    nc.vector.tensor_copy(
                        out=ob[:, bass_ts(j)], in_=pos[j][:])
                nc.sync.dma_start(
                    out=out[mb * P:(mb + 1) * P,
                            half * (N_SH // 2):(half + 1) * (N_SH // 2)],
                    in_=ob[:])

            # DMA issue order: a couple of x blocks, the first column-half
            # of w, the remaining x blocks, then the second half of w.  Its
            # signs land in the dense phase, where ACT/DVE have slack.
            load_x(0)
            for kb2 in range(KB // 2):
                load_w(kb2, 0)
                if kb2 == 3:
                    load_x(1)
            for mb in range(X_HEAD, MB):
                load_x(mb)
            for kb2 in range(KB // 2):
                load_w(kb2, 1)

            # Compute in two sweeps over the row blocks, one per w
            # column-half; sweep 1 starts as soon as the first half of w is
            # signed, sweep 2 as the second half lands behind it.
            for mb in range(TLOOK):
                transposes(mb)
            for mb in range(MB):
                matmuls(mb, 0, surf=(mb == 0))
                if mb + TLOOK < MB:
                    transposes(mb + TLOOK)
            for mb in range(MB):
                matmuls(mb, 1)

    nc.compile()
    return nc


def get_nc():
    global _NC_CACHE
    if _NC_CACHE is None:
        _NC_CACHE = build_nc()
    return _NC_CACHE


def kernel(x: np.ndarray, w: np.ndarray) -> np.ndarray:
    x = np.asarray(x, dtype=np.float32)
    w = np.asarray(w, dtype=np.float32)
    assert x.shape == (M_FULL, K) and w.shape == (K, N_FULL)

    nc = get_nc()
    in_maps = []
    for c in range(N_CORES):
        mi, ni = divmod(c, RN)
        in_maps.append({
            "x": np.ascontiguousarray(x[mi * M_SH:(mi + 1) * M_SH, :]),
            "w": np.ascontiguousarray(w[:, ni * N_SH:(ni + 1) * N_SH]),
        })
    res = run_bass_kernel_spmd(nc, in_maps, list(range(N_CORES)))

    out = np.empty((M_FULL, N_FULL), dtype=np.float32)
    for c in range(N_CORES):
        mi, ni = divmod(c, RN)
        out[mi * M_SH:(mi + 1) * M_SH, ni * N_SH:(ni + 1) * N_SH] = \
            res.results[c]["out"].astype(np.float32)
    return out
